# revision 1
# baseline (speedup 1.0000x reference)
"""Trainium2 Bass kernel for ConvPolicy14 (dense_cnn, 93 -> 40 policy net).

Strategy: the network is tiny (~4.6K MACs), so every conv/pool/upsample/concat
is folded (input-independently, on host) into a chain of 8 small dense affine
layers executed as TensorE matmuls with PSUM accumulation:

    h1 = tanh(M1 v0 + b1)            v0 = jcat flattened (84)
    h2 = tanh(M2 h1 + b2)
    h3 = tanh(M3 h2 + b3)            M3 = conv3_toeplitz @ avgpool
    h4 = tanh(M4 h3 + b4)
    h5 = tanh(M5 (h4 + ext) + b5)    ext = [psi, x47, x52]; split into 2 matmuls
    h6 = tanh(M6a h5 + M6b h3 + b6)  concat -> accumulating matmul pair
    h7 = tanh(M7a h6 + M7b h2 + b7)  M7a folds nearest-upsample
    out = M8a h7 + M8b v0 + b8       (40,) final, no tanh

Biases are folded into the matmul accumulation groups as extra contraction
rows against constant-1.0 cells (v0 has structural zeros reused as the 1.0
slot for layers 1/8), so ScalarE does pure tanh with no operand loads and
every instruction carries at most one sync wait (walrus S3_LW limit).
psi = atan2(qz,qw) - atan2(-qx,qy) is computed on-device (DVE reciprocal /
compares + one ACT Arctan, branchless quadrant fix) off the critical path.
All ACT functions used (Tanh/Arctan/Copy) live in one table set so a single
~2.7us ACT table load is paid. One input DMA, one output DMA.

Per the sharding hint the problem is too small to shard: all 8 cores run the
same program; core 0's output is returned.
"""

import numpy as np

F32 = np.float32

# ---------------------------------------------------------------------------
# Block layout (single DMA'd constant/input block, 128 partitions x _B_COLS)
# ---------------------------------------------------------------------------
_C_L1 = 0        # lhsT1 (84, 28); row0 = b1 (v0[0] := 1.0 trick)
_C_L2 = 28       # lhsT2 (28, 28)
_C_B2 = 56       # b2 row (1, 28)
_C_L3 = 84       # lhsT3 (28, 12)
_C_B3 = 96       # b3 row (1, 12)
_C_L4 = 108      # lhsT4 (12, 3)
_C_B4 = 111      # b4 row (1, 3)
_C_L5A = 114     # lhsT5a (3, 12)
_C_L5B = 126     # lhsT5b (4, 12); row3 = b5
_C_L6A = 138     # lhsT6a (12, 12)
_C_L6B = 150     # lhsT6b (12, 12)
_C_B6 = 162      # b6 row (1, 12)
_C_L7A = 174     # lhsT7a (12, 28)
_C_L7B = 202     # lhsT7b (28, 28)
_C_B7 = 230      # b7 row (1, 28)
_C_L8A = 258     # lhsT8a (28, 40)
_C_L8B = 298     # lhsT8b (84, 40); row0 = b8
_XO = 338        # x block start
_C_V0 = _XO + 0      # v0 col (p0 = 1.0, p2-41 = x[7:47], p44-83 = x[53:93])
_C_EXT = _XO + 1     # ext col (p0 = psi slot, p1 = x47, p2 = x52, p3 = 1.0)
_C_DEN = _XO + 2     # [qw, qy] at p0, 2 cols
_C_NUM = _XO + 4     # [qz, qx] at p0, 2 cols
_C_ONE = _XO + 6     # 1.0 at p0
_B_COLS = _XO + 8


def _toeplitz_conv(cw, L):
    """Conv1d pad=1 k=3: out[(o,l)] = sum_{c,k} cw[o,c,k] x[(c,l+k-1)]."""
    O, C, _ = cw.shape
    M = np.zeros((O * L, C * L), F32)
    for o in range(O):
        for l in range(L):
            for c in range(C):
                for k in range(3):
                    m = l + k - 1
                    if 0 <= m < L:
                        M[o * L + l, c * L + m] = cw[o, c, k]
    return M


def _toeplitz_deconv(dw, L):
    """ConvTranspose1d pad=1 k=3 s=1: out[(o,l)] = sum dw[c,o,1-m+l] x[(c,m)]."""
    C, O, _ = dw.shape
    M = np.zeros((O * L, C * L), F32)
    for o in range(O):
        for l in range(L):
            for c in range(C):
                for m in range(L):
                    k = 1 - m + l
                    if 0 <= k < 3:
                        M[o * L + l, c * L + m] = dw[c, o, k]
    return M


def _build_w_block(w):
    c1w, c1b = w["conv1_w"], w["conv1_b"]
    c2w, c2b = w["conv2_w"], w["conv2_b"]
    c3w, c3b = w["conv3_w"], w["conv3_b"]
    c4w, c4b = w["conv4_w"], w["conv4_b"]
    d1w, d1b = w["deconv1_w"], w["deconv1_b"]
    d2w, d2b = w["deconv2_w"], w["deconv2_b"]
    d3w, d3b = w["deconv3_w"], w["deconv3_b"]
    d4w, d4b = w["deconv4_w"], w["deconv4_b"]

    M1 = _toeplitz_conv(c1w, 7)                     # (28, 84)
    M2 = _toeplitz_conv(c2w, 7)                     # (28, 28)

    # adaptive avg pool (4,7)->(4,3), windows [0:3],[2:5],[4:7]
    P = np.zeros((12, 28), F32)
    for c in range(4):
        for j in range(3):
            P[c * 3 + j, c * 7 + 2 * j: c * 7 + 2 * j + 3] = 1.0 / 3.0
    T3 = np.zeros((12, 12), F32)
    for o in range(4):
        for j in range(3):
            for c in range(4):
                for k in range(3):
                    jp = j + k - 1
                    if 0 <= jp < 3:
                        T3[o * 3 + j, c * 3 + jp] = c3w[o, c, k]
    M3 = (T3.astype(np.float64) @ P.astype(np.float64)).astype(F32)  # (12, 28)

    M4 = np.zeros((3, 12), F32)                     # conv4 pad0 L3->1
    for o in range(3):
        for c in range(4):
            M4[o, c * 3: c * 3 + 3] = c4w[o, c, :]

    M5 = np.zeros((12, 3), F32)                     # deconv1 L1->3
    for o in range(4):
        for l in range(3):
            for c in range(3):
                M5[o * 3 + l, c] = d1w[c, o, l]

    M6 = _toeplitz_deconv(d2w, 3)                   # (12, 24)

    T7 = _toeplitz_deconv(d3w, 7)                   # (28, 56)
    g = [0, 0, 0, 1, 1, 2, 2]                       # nearest-upsample 3->7
    U = np.zeros((28, 12), F32)
    for c in range(4):
        for l in range(7):
            U[c * 7 + l, c * 3 + g[l]] = 1.0
    M7a = (T7[:, :28].astype(np.float64) @ U.astype(np.float64)).astype(F32)
    M7b = np.ascontiguousarray(T7[:, 28:])          # (28, 28)

    M8 = _toeplitz_deconv(d4w, 7)[2:, :]            # (40, 112): acts[2:]
    b8 = np.repeat(d4b, 7).astype(F32)[2:]

    b1 = np.repeat(c1b, 7).astype(F32)
    b2 = np.repeat(c2b, 7).astype(F32)
    b3 = np.repeat(c3b, 3).astype(F32)
    b5 = np.repeat(d1b, 3).astype(F32)
    b6 = np.repeat(d2b, 3).astype(F32)
    b7 = np.repeat(d3b, 7).astype(F32)

    # v0[0] / v0[1] are structural zeros; v0[0] is repurposed as a 1.0 cell.
    # Zero the (irrelevant) weight columns and plant biases there.
    lhsT1 = M1.T.copy()                 # (84, 28)
    lhsT1[0, :] = b1
    lhsT1[1, :] = 0.0
    M8b = M8[:, 28:].copy()             # (40, 84) weights on v0
    lhsT8b = M8b.T.copy()               # (84, 40)
    lhsT8b[0, :] = b8
    lhsT8b[1, :] = 0.0
    lhsT5b = np.concatenate([M5.T, b5[None, :]], axis=0)  # (4, 12)

    blk = np.zeros((128, _B_COLS), F32)

    def put(col, mat):
        K, M = mat.shape
        blk[:K, col:col + M] = mat

    put(_C_L1, lhsT1)
    put(_C_L2, M2.T)
    put(_C_B2, b2[None, :])
    put(_C_L3, M3.T)
    put(_C_B3, b3[None, :])
    put(_C_L4, M4.T)
    put(_C_B4, np.asarray(c4b, F32)[None, :])
    put(_C_L5A, M5.T)
    put(_C_L5B, lhsT5b)
    put(_C_L6A, M6[:, :12].T)
    put(_C_L6B, M6[:, 12:].T)
    put(_C_B6, b6[None, :])
    put(_C_L7A, M7a.T)
    put(_C_L7B, M7b.T)
    put(_C_B7, b7[None, :])
    put(_C_L8A, M8[:, :28].T)
    put(_C_L8B, lhsT8b)
    return blk


def _fill_x_block(blk, x):
    x = np.asarray(x, F32).reshape(-1)
    blk[:, _XO:] = 0.0
    blk[0, _C_V0] = 1.0                 # constant-1 slot (v0[0] structural 0)
    blk[2:42, _C_V0] = x[7:47]
    blk[44:84, _C_V0] = x[53:93]
    blk[1, _C_EXT] = x[47]              # p0 left 0: psi computed on device
    blk[2, _C_EXT] = x[52]
    blk[3, _C_EXT] = 1.0                # bias slot for layer 5
    blk[0, _C_DEN] = x[3]               # qw
    blk[0, _C_DEN + 1] = x[5]           # qy
    blk[0, _C_NUM] = x[6]               # qz
    blk[0, _C_NUM + 1] = x[4]           # qx
    blk[0, _C_ONE] = 1.0
    return blk


_CACHE = {}


def _build_bass():
    if "nc" in _CACHE:
        return _CACHE["nc"]

    import concourse.mybir as mybir
    from concourse import bacc, tile

    f32 = mybir.dt.float32
    AF = mybir.ActivationFunctionType
    OP = mybir.AluOpType

    class _OneSetBacc(bacc.Bacc):
        """Force every activation to resolve to sigmoid_and_others (it covers
        Tanh/Arctan/Copy/Identity) so only one ~2.7us ACT table load is paid.
        Canonical set order (= act_func_set_id) is preserved; other sets just
        stop advertising the functions we use."""

        def insert_act_table_loads(self):
            import bass_rust as _bass_rust
            from concourse.hw_specs import get_activation_tables

            has_activation = any(
                isinstance(i, mybir.InstActivation)
                for b in self.main_func.blocks
                for i in b.instructions
            )
            if not has_activation:
                return
            tables = list(get_activation_tables(self.m.arch).items())
            ours = dict(tables)["sigmoid_and_others"]
            for f in (AF.Tanh, AF.Arctan, AF.Copy, AF.Identity):
                assert f in ours, f
            tables = [(n, (fns if n == "sigmoid_and_others" else fns - ours))
                      for n, fns in tables]
            _bass_rust.insert_act_table_loads(self, tables)

    class _SlimTile(tile.TileContext):
        """Replace Tile's kernel tail (2 all-engine barriers + DMA-ring reset,
        ~5us on HW) with: gated drain -> gpsimd range sem_clear. The drain
        already waits on every proc's final tick, so clearing is safe once it
        completes; sems still end at 0 for re-execution."""

        def _drain_and_barrier(self, tick_clock, wait_clock):
            from concourse.vector_clock import ScopedClock
            from concourse.bass import compact_to_ranges

            nc = self.nc
            drain_inst = nc.sync.drain()
            wait_clock.add_sem_waits(
                drain_inst.ins, ScopedClock({None: tick_clock.global_clock})
            )
            done = nc.alloc_semaphore(f"slim_done_{nc.next_id()}")
            drain_inst.then_inc(done)
            popped = nc._tile_sem_poison_stack.pop()
            assert popped is self._sem_poison
            nc.gpsimd.wait_ge(done, 1)
            sems = list(self.sems.allocated().values())
            sem_nums = [s.num if hasattr(s, "num") else int(s) for s in sems]
            sem_nums.append(done.num)
            for r in compact_to_ranges(sorted(sem_nums)):
                nc.gpsimd.sem_clear(r)
            nc._state.prepend_free_semaphores(sem_nums)
            for ps in nc._tile_sem_poison_stack:
                ps.update(sem_nums)

    nc = _OneSetBacc("TRN2", num_devices=8)
    b_dram = nc.declare_dram_parameter("blk", [128, _B_COLS], f32, isOutput=False)
    out_dram = nc.declare_dram_parameter("out", [40, 1], f32, isOutput=True)

    with _SlimTile(nc) as tc:
        with (
            tc.tile_pool(name="sbuf", bufs=1) as pool,
            tc.tile_pool(name="psum", bufs=1, space="PSUM") as psum,
        ):
            Bt = pool.tile([128, _B_COLS], f32, name="Bt", tag="bt")
            Ht = pool.tile([128, 9], f32, name="Ht", tag="ht")
            St = pool.tile([128, 24], f32, name="St", tag="st")

            nc.sync.dma_start(Bt[:, :], b_dram[:, :])

            dims = [28, 28, 12, 3, 12, 12, 28, 40]
            ps = [psum.tile([m, 1], f32, name=f"ps{i}", tag=f"ps{i}")
                  for i, m in enumerate(dims)]

            def mm(i, lhs_col, k, m, rhs, start, stop):
                nc.tensor.matmul(ps[i][:, :], Bt[0:k, lhs_col:lhs_col + m],
                                 rhs, start=start, stop=stop)

            def tanh_to(i, m, hcol, func=AF.Tanh):
                nc.scalar.activation(Ht[0:m, hcol:hcol + 1], ps[i][0:m, 0:1],
                                     func, bias=0.0, scale=1.0)

            one = Bt[0:1, _C_ONE:_C_ONE + 1]
            EXTC = 7   # Ht col: ext vector [psi, x47, x52, 1.0]
            OUTC = 8   # Ht col: final output

            # --- atan2 DVE stage (ready right after the DMA). q is emitted
            # LAST so arctan's single DVE wait covers the whole chain. ---
            nc.vector.reciprocal(St[0:1, 0:2], Bt[0:1, _C_DEN:_C_DEN + 2])
            nc.vector.tensor_scalar(St[0:1, 4:6], Bt[0:1, _C_DEN:_C_DEN + 2],
                                    0.0, None, OP.is_lt)            # [den<0]
            nc.vector.tensor_scalar(St[0:1, 6:8], Bt[0:1, _C_NUM:_C_NUM + 2],
                                    0.0, None, OP.is_ge)            # [num>=0]
            nc.vector.tensor_scalar(St[0:1, 8:10], St[0:1, 6:8],
                                    2.0, -1.0, OP.mult, OP.add)     # sign(num)
            nc.vector.tensor_mul(St[0:1, 10:12], St[0:1, 8:10], St[0:1, 4:6])
            nc.vector.tensor_scalar(St[0:1, 12:14], St[0:1, 10:12],
                                    float(np.pi / 2), None, OP.mult)
            nc.vector.tensor_add(St[0:1, 14:15], St[0:1, 12:13],
                                 St[0:1, 13:14])                    # b = corr/2
            nc.vector.tensor_mul(St[0:1, 2:4], Bt[0:1, _C_NUM:_C_NUM + 2],
                                 St[0:1, 0:2])                      # q = num/den

            # --- L1 (bias in lhsT1 row 0 against v0[0]=1.0) ---
            mm(0, _C_L1, 84, 28, Bt[0:84, _C_V0:_C_V0 + 1], True, True)
            tanh_to(0, 28, 0)
            # ext tail [x47, x52, 1.0] -> Ht; also gives ACT its one DMA wait
            nc.scalar.activation(Ht[0:4, EXTC:EXTC + 1],
                                 Bt[0:4, _C_EXT:_C_EXT + 1], AF.Copy,
                                 bias=0.0, scale=1.0)
            # --- atan2 ACT tail, filling the tanh1->tanh2 gap:
            # psi = sum_f(arctan(q_f) + b) via Identity + accum_out.
            # b is bounced through an ACT copy so the psi op's waits stay
            # single-proc (walrus allows one sync wait per instruction). ---
            nc.scalar.activation(St[0:1, 20:21], St[0:1, 14:15], AF.Copy,
                                 bias=0.0, scale=1.0)
            nc.scalar.activation(St[0:1, 16:18], St[0:1, 2:4], AF.Arctan,
                                 bias=0.0, scale=1.0)
            nc.scalar.activation(St[0:1, 18:20], St[0:1, 16:18], AF.Identity,
                                 bias=St[0:1, 20:21], scale=1.0,
                                 accum_out=Ht[0:1, EXTC:EXTC + 1])

            # --- L2 ---
            mm(1, _C_B2, 1, 28, one, True, False)
            mm(1, _C_L2, 28, 28, Ht[0:28, 0:1], False, True)
            tanh_to(1, 28, 1)

            # --- L3 ---
            mm(2, _C_B3, 1, 12, one, True, False)
            mm(2, _C_L3, 28, 12, Ht[0:28, 1:2], False, True)
            tanh_to(2, 12, 2)
            # --- L4 ---
            mm(3, _C_B4, 1, 3, one, True, False)
            mm(3, _C_L4, 12, 3, Ht[0:12, 2:3], False, True)
            tanh_to(3, 3, 3)
            # --- L5: M5 (h4 + ext) + b5 ---
            mm(4, _C_L5B, 4, 12, Ht[0:4, EXTC:EXTC + 1], True, False)
            mm(4, _C_L5A, 3, 12, Ht[0:3, 3:4], False, True)
            tanh_to(4, 12, 4)
            # --- L6 ---
            mm(5, _C_B6, 1, 12, one, True, False)
            mm(5, _C_L6B, 12, 12, Ht[0:12, 2:3], False, False)
            mm(5, _C_L6A, 12, 12, Ht[0:12, 4:5], False, True)
            tanh_to(5, 12, 5)
            # --- L7 ---
            mm(6, _C_B7, 1, 28, one, True, False)
            mm(6, _C_L7B, 28, 28, Ht[0:28, 1:2], False, False)
            mm(6, _C_L7A, 12, 28, Ht[0:12, 5:6], False, True)
            tanh_to(6, 28, 6)
            # --- L8 (bias in lhsT8b row 0; no tanh) ---
            mm(7, _C_L8B, 84, 40, Bt[0:84, _C_V0:_C_V0 + 1], True, False)
            mm(7, _C_L8A, 28, 40, Ht[0:28, 6:7], False, True)
            tanh_to(7, 40, OUTC, func=AF.Copy)

            nc.sync.dma_start(out_dram[:, :], Ht[0:40, OUTC:OUTC + 1])

    nc.compile()
    _CACHE["nc"] = nc
    return nc


def _build_blk(inputs):
    blk = _build_w_block(inputs)
    _fill_x_block(blk, inputs["x"])
    return blk


def kernel(**inputs) -> np.ndarray:
    nc = _build_bass()
    blk = _build_blk(inputs)

    from concourse.bass_utils import run_bass_kernel_spmd

    res = run_bass_kernel_spmd(nc, [{"blk": blk.copy()} for _ in range(8)],
                               core_ids=list(range(8)))
    out = np.asarray(res.results[0]["out"], F32).reshape(1, 40)
    return out



# revision 2
# speedup vs baseline: 1.3375x; 1.3375x over previous
"""Trainium2 Bass kernel for ConvPolicy14 (dense_cnn, 93 -> 40 policy net).

Strategy: every conv/pool/upsample/concat is folded (input-independently, on
host) into a chain of 8 small dense affine layers executed as TensorE matmuls
with PSUM accumulation:

    h1 = tanh(M1 v0 + b1)            v0 = jcat flattened (84)
    h2 = tanh(M2 h1 + b2)
    h3 = tanh(M3 h2 + b3)            M3 = conv3_toeplitz @ avgpool
    h4 = tanh(M4 h3 + b4)
    h5 = tanh(M5 (h4 + ext) + b5)    ext = [psi, x47, x52]
    h6 = tanh(M6a h5 + M6b h3 + b6)
    h7 = tanh(M7a h6 + M7b h2 + b7)  M7a folds nearest-upsample
    out = M8a h7 + M8b v0 + b8       (1, 40) final, no tanh

Layout/scheduling choices (vs the 21.7-24.5us ancestor):
  * One SBUF block Bt[84, 279] holds ALL weights + x-cells + the h-state
    columns. Biases ride as extra contraction rows against 1.0 cells that the
    input DMA plants at h-col row m (so there are NO separate bias matmuls and
    no extra sem waits -- the DMA write is covered transitively).
  * The final layer is computed TRANSPOSED (h7/v0 as the stationary 1-col
    lhsT, the weight blocks as the moving rhs) so the result lands as a
    [1, 40] PSUM row -> one contiguous 160B output DMA descriptor instead of
    a 40-partition x 4B scatter (which cost ~4.6us on HW).
  * Two input DMAs: a 40-col block (lhsT1 + x cells + h cols) on the SP
    queue lands early so mm1 starts sooner; the remaining 239 weight cols go
    concurrently through the gpsimd SWDGE queue.
  * psi = atan2(qz,qw) - atan2(-qx,qy): DVE computes q = num/den and the
    quadrant correction c = sign(num)*pi*[den<0] (6 ops); ACT does ONE
    Arctan writing [at0, at1] straight into the ext column; c lands in a
    second column. The L5 matmul picks up psi = at0+at1+c0+c1 by replicating
    M5's psi-row across those 4 contraction cells. No accumulator read, no
    bounce copies.
  * Output DMA is issued by the ACT engine right after the PSUM->SBUF copy
    (no cross-engine hop).
  * One ~2.7us ACT table load (Tanh/Arctan/Copy forced into one set);
    Tile's 2-barrier teardown replaced by gated drain + gpsimd sem clear.

Per the sharding hint the problem is too small to shard: all 8 cores run the
same program; core 0's output is returned.
"""

import numpy as np

F32 = np.float32

# ---------------------------------------------------------------------------
# Block layout: 84 partitions x _B_COLS, split into two DMAs at col _SPLIT
# ---------------------------------------------------------------------------
_C_L1 = 0        # lhsT1 (84, 28); row0 = b1, row1 = 0 (v0[0] := 1.0 trick)
_C_V0 = 28       # v0 col (p0 = 1.0, p2-41 = x[7:47], p44-83 = x[53:93])
_C_EXT = 29      # ext col (p0,p1 = arctan outs; p2 = x47, p3 = x52, p4 = 1.0)
_C_DEN = 30      # [qw, qy] at p0,p1
_C_NUM = 31      # [qz, qx] at p0,p1
_C_EXT2 = 32     # quadrant-correction col (p0,p1 written by DVE)
_C_H = 33        # h1..h7 state cols (33..39); 1.0 bias cells planted by DMA
_SPLIT = 40      # DMA1 = cols [0, 40), DMA2 = cols [40, _B_COLS)
_C_L2 = 40       # lhsT2 (29, 28); row28 = b2
_C_L3 = 68       # lhsT3 (29, 12); row28 = b3
_C_L4 = 80       # lhsT4 (13, 3);  row12 = b4
_C_L5B = 83      # lhsT5b (5, 12): [psi; psi; x47; x52; b5] rows
_C_L5C = 95      # lhsT5c (2, 12): [psi; psi] rows (vs correction col)
_C_L5A = 107     # lhsT5a (3, 12) = M5^T (vs h4)
_C_L6B = 119     # lhsT6b (12, 12) (vs h3)
_C_L6A = 131     # lhsT6a (13, 12); row12 = b6 (vs h5)
_C_L7B = 143     # lhsT7b (28, 28) (vs h2)
_C_L7A = 171     # lhsT7a (13, 28); row12 = b7 (vs h6)
_C_L8A = 199     # rhs8a (28, 40) = M8a^T (moving operand, vs h7 stationary)
_C_L8B = 239     # rhs8b (84, 40); row0 = b8, row1 = 0 (vs v0 stationary)
_B_COLS = 279


def _toeplitz_conv(cw, L):
    """Conv1d pad=1 k=3: out[(o,l)] = sum_{c,k} cw[o,c,k] x[(c,l+k-1)]."""
    O, C, _ = cw.shape
    M = np.zeros((O * L, C * L), F32)
    for o in range(O):
        for l in range(L):
            for c in range(C):
                for k in range(3):
                    m = l + k - 1
                    if 0 <= m < L:
                        M[o * L + l, c * L + m] = cw[o, c, k]
    return M


def _toeplitz_deconv(dw, L):
    """ConvTranspose1d pad=1 k=3 s=1: out[(o,l)] = sum dw[c,o,1-m+l] x[(c,m)]."""
    C, O, _ = dw.shape
    M = np.zeros((O * L, C * L), F32)
    for o in range(O):
        for l in range(L):
            for c in range(C):
                for m in range(L):
                    k = 1 - m + l
                    if 0 <= k < 3:
                        M[o * L + l, c * L + m] = dw[c, o, k]
    return M


def _build_w_block(w):
    c1w, c1b = w["conv1_w"], w["conv1_b"]
    c2w, c2b = w["conv2_w"], w["conv2_b"]
    c3w, c3b = w["conv3_w"], w["conv3_b"]
    c4w, c4b = w["conv4_w"], w["conv4_b"]
    d1w, d1b = w["deconv1_w"], w["deconv1_b"]
    d2w, d2b = w["deconv2_w"], w["deconv2_b"]
    d3w, d3b = w["deconv3_w"], w["deconv3_b"]
    d4w, d4b = w["deconv4_w"], w["deconv4_b"]

    M1 = _toeplitz_conv(c1w, 7)                     # (28, 84)
    M2 = _toeplitz_conv(c2w, 7)                     # (28, 28)

    # adaptive avg pool (4,7)->(4,3), windows [0:3],[2:5],[4:7]
    P = np.zeros((12, 28), F32)
    for c in range(4):
        for j in range(3):
            P[c * 3 + j, c * 7 + 2 * j: c * 7 + 2 * j + 3] = 1.0 / 3.0
    T3 = np.zeros((12, 12), F32)
    for o in range(4):
        for j in range(3):
            for c in range(4):
                for k in range(3):
                    jp = j + k - 1
                    if 0 <= jp < 3:
                        T3[o * 3 + j, c * 3 + jp] = c3w[o, c, k]
    M3 = (T3.astype(np.float64) @ P.astype(np.float64)).astype(F32)  # (12, 28)

    M4 = np.zeros((3, 12), F32)                     # conv4 pad0 L3->1
    for o in range(3):
        for c in range(4):
            M4[o, c * 3: c * 3 + 3] = c4w[o, c, :]

    M5 = np.zeros((12, 3), F32)                     # deconv1 L1->3
    for o in range(4):
        for l in range(3):
            for c in range(3):
                M5[o * 3 + l, c] = d1w[c, o, l]

    M6 = _toeplitz_deconv(d2w, 3)                   # (12, 24)

    T7 = _toeplitz_deconv(d3w, 7)                   # (28, 56)
    g = [0, 0, 0, 1, 1, 2, 2]                       # nearest-upsample 3->7
    U = np.zeros((28, 12), F32)
    for c in range(4):
        for l in range(7):
            U[c * 7 + l, c * 3 + g[l]] = 1.0
    M7a = (T7[:, :28].astype(np.float64) @ U.astype(np.float64)).astype(F32)
    M7b = np.ascontiguousarray(T7[:, 28:])          # (28, 28)

    M8 = _toeplitz_deconv(d4w, 7)[2:, :]            # (40, 112): acts[2:]
    b8 = np.repeat(d4b, 7).astype(F32)[2:]

    b1 = np.repeat(c1b, 7).astype(F32)
    b2 = np.repeat(c2b, 7).astype(F32)
    b3 = np.repeat(c3b, 3).astype(F32)
    b5 = np.repeat(d1b, 3).astype(F32)
    b6 = np.repeat(d2b, 3).astype(F32)
    b7 = np.repeat(d3b, 7).astype(F32)

    blk = np.zeros((84, _B_COLS), F32)

    def put(col, mat):
        K, M = mat.shape
        blk[:K, col:col + M] = mat

    # v0[0] / v0[1] are structural zeros; v0[0] is repurposed as a 1.0 cell.
    lhsT1 = M1.T.copy()                 # (84, 28)
    lhsT1[0, :] = b1
    lhsT1[1, :] = 0.0
    put(_C_L1, lhsT1)

    put(_C_L2, np.vstack([M2.T, b2[None, :]]))              # (29, 28)
    put(_C_L3, np.vstack([M3.T, b3[None, :]]))              # (29, 12)
    put(_C_L4, np.vstack([M4.T, np.asarray(c4b, F32)[None, :]]))  # (13, 3)
    psi_row = np.ascontiguousarray(M5[:, 0])[None, :]       # (1, 12)
    put(_C_L5B, np.vstack([psi_row, psi_row, M5[:, 1][None, :],
                           M5[:, 2][None, :], b5[None, :]]))  # (5, 12)
    put(_C_L5C, np.vstack([psi_row, psi_row]))              # (2, 12)
    put(_C_L5A, M5.T)                                       # (3, 12)
    put(_C_L6B, M6[:, 12:].T)                               # (12, 12)
    put(_C_L6A, np.vstack([M6[:, :12].T, b6[None, :]]))     # (13, 12)
    put(_C_L7B, M7b.T)                                      # (28, 28)
    put(_C_L7A, np.vstack([M7a.T, b7[None, :]]))            # (13, 28)
    put(_C_L8A, M8[:, :28].T)                               # (28, 40)
    rhs8b = M8[:, 28:].T.copy()                             # (84, 40)
    rhs8b[0, :] = b8
    rhs8b[1, :] = 0.0
    put(_C_L8B, rhs8b)

    # 1.0 bias cells read through the aug rows of lhsT2/3/4/6a/7a
    blk[28, _C_H + 0] = 1.0     # h1 col, read [0:29] by mm2
    blk[28, _C_H + 1] = 1.0     # h2 col, read [0:29] by mm3
    blk[12, _C_H + 2] = 1.0     # h3 col, read [0:13] by mm4
    blk[12, _C_H + 4] = 1.0     # h5 col, read [0:13] by mm6b
    blk[12, _C_H + 5] = 1.0     # h6 col, read [0:13] by mm7b
    return blk


def _fill_x_block(blk, x):
    x = np.asarray(x, F32).reshape(-1)
    blk[:, _C_V0:_C_H + 7] = 0.0
    blk[28, _C_H + 0] = 1.0
    blk[28, _C_H + 1] = 1.0
    blk[12, _C_H + 2] = 1.0
    blk[12, _C_H + 4] = 1.0
    blk[12, _C_H + 5] = 1.0
    blk[0, _C_V0] = 1.0                 # constant-1 slot (v0[0] structural 0)
    blk[2:42, _C_V0] = x[7:47]
    blk[44:84, _C_V0] = x[53:93]
    blk[2, _C_EXT] = x[47]              # p0,p1 left 0: arctan lands there
    blk[3, _C_EXT] = x[52]
    blk[4, _C_EXT] = 1.0                # bias slot for layer 5
    blk[0, _C_DEN] = x[3]               # qw
    blk[1, _C_DEN] = x[5]               # qy
    blk[0, _C_NUM] = x[6]               # qz
    blk[1, _C_NUM] = x[4]               # qx
    return blk


_CACHE = {}


def _build_bass():
    if "nc" in _CACHE:
        return _CACHE["nc"]

    import concourse.mybir as mybir
    from concourse import bacc, tile

    f32 = mybir.dt.float32
    AF = mybir.ActivationFunctionType
    OP = mybir.AluOpType
    PI = float(np.pi)

    class _OneSetBacc(bacc.Bacc):
        """Force every activation to resolve to sigmoid_and_others (it covers
        Tanh/Arctan/Copy) so only one ~2.7us ACT table load is paid."""

        def insert_act_table_loads(self):
            import bass_rust as _bass_rust
            from concourse.hw_specs import get_activation_tables

            has_activation = any(
                isinstance(i, mybir.InstActivation)
                for b in self.main_func.blocks
                for i in b.instructions
            )
            if not has_activation:
                return
            tables = list(get_activation_tables(self.m.arch).items())
            ours = dict(tables)["sigmoid_and_others"]
            for f in (AF.Tanh, AF.Arctan, AF.Copy):
                assert f in ours, f
            tables = [(n, (fns if n == "sigmoid_and_others" else fns - ours))
                      for n, fns in tables]
            _bass_rust.insert_act_table_loads(self, tables)

    class _SlimTile(tile.TileContext):
        """Replace Tile's kernel tail (2 all-engine barriers + DMA-ring reset,
        ~5us on HW) with: gated drain -> gpsimd range sem_clear."""

        def _drain_and_barrier(self, tick_clock, wait_clock):
            from concourse.vector_clock import ScopedClock
            from concourse.bass import compact_to_ranges

            nc = self.nc
            drain_inst = nc.sync.drain()
            wait_clock.add_sem_waits(
                drain_inst.ins, ScopedClock({None: tick_clock.global_clock})
            )
            done = nc.alloc_semaphore(f"slim_done_{nc.next_id()}")
            drain_inst.then_inc(done)
            popped = nc._tile_sem_poison_stack.pop()
            assert popped is self._sem_poison
            nc.gpsimd.wait_ge(done, 1)
            sems = list(self.sems.allocated().values())
            sem_nums = [s.num if hasattr(s, "num") else int(s) for s in sems]
            sem_nums.append(done.num)
            for r in compact_to_ranges(sorted(sem_nums)):
                nc.gpsimd.sem_clear(r)
            nc._state.prepend_free_semaphores(sem_nums)
            for ps in nc._tile_sem_poison_stack:
                ps.update(sem_nums)

    nc = _OneSetBacc("TRN2", num_devices=8)
    b1_dram = nc.declare_dram_parameter("blk1", [84, _SPLIT], f32,
                                        isOutput=False)
    b2_dram = nc.declare_dram_parameter("blk2", [84, _B_COLS - _SPLIT], f32,
                                        isOutput=False)
    out_dram = nc.declare_dram_parameter("out", [1, 40], f32, isOutput=True)

    with _SlimTile(nc) as tc:
        with (
            tc.tile_pool(name="sbuf", bufs=1) as pool,
            tc.tile_pool(name="psum", bufs=1, space="PSUM") as psum,
        ):
            Bt = pool.tile([84, _B_COLS], f32, name="Bt", tag="bt")
            St = pool.tile([8, 48], f32, name="St", tag="st")

            # DMA1 (x + L1 + h cols) on SP; DMA2 (remaining weights) on the
            # gpsimd SWDGE queue so the two transfers overlap.
            nc.sync.dma_start(Bt[:, 0:_SPLIT], b1_dram[:, :])
            nc.gpsimd.dma_start(Bt[:, _SPLIT:_B_COLS], b2_dram[:, :])

            dims = [28, 28, 12, 3, 12, 12, 28]
            ps = [psum.tile([m, 1], f32, name=f"ps{i}", tag=f"ps{i}")
                  for i, m in enumerate(dims)]
            ps8 = psum.tile([1, 40], f32, name="ps8", tag="ps8")

            H = _C_H

            def mm(i, lhs_col, k, m, rhs, start, stop):
                nc.tensor.matmul(ps[i][:, :], Bt[0:k, lhs_col:lhs_col + m],
                                 rhs, start=start, stop=stop)

            def tanh_to(i, m, hcol):
                nc.scalar.activation(Bt[0:m, hcol:hcol + 1], ps[i][0:m, 0:1],
                                     AF.Tanh, bias=0.0, scale=1.0)

            # --- psi DVE stage: q = num/den last-but-one, correction c
            # lands directly in the EXT2 col (covered vs DMA1 via recip). ---
            nc.vector.reciprocal(St[0:2, 0:1], Bt[0:2, _C_DEN:_C_DEN + 1])
            nc.vector.tensor_scalar(St[0:2, 2:3], Bt[0:2, _C_DEN:_C_DEN + 1],
                                    0.0, None, OP.is_lt)        # [den<0]
            nc.vector.tensor_scalar(St[0:2, 3:4], Bt[0:2, _C_NUM:_C_NUM + 1],
                                    0.0, None, OP.is_ge)        # [num>=0]
            nc.vector.tensor_mul(St[0:2, 1:2], Bt[0:2, _C_NUM:_C_NUM + 1],
                                 St[0:2, 0:1])                  # q = num/den
            nc.vector.tensor_scalar(St[0:2, 4:5], St[0:2, 3:4],
                                    2.0 * PI, -PI, OP.mult, OP.add)  # sign*pi
            nc.vector.tensor_mul(Bt[0:2, _C_EXT2:_C_EXT2 + 1],
                                 St[0:2, 4:5], St[0:2, 2:3])    # c

            # --- psi ACT stage: one Arctan, straight into the ext col ---
            nc.scalar.activation(Bt[0:2, _C_EXT:_C_EXT + 1], St[0:2, 1:2],
                                 AF.Arctan, bias=0.0, scale=1.0)

            # --- L1 (bias in lhsT1 row 0 against v0[0]=1.0) ---
            mm(0, _C_L1, 84, 28, Bt[0:84, _C_V0:_C_V0 + 1], True, True)
            tanh_to(0, 28, H + 0)
            # --- L2..L4: single matmuls, bias rows against DMA-planted 1.0 ---
            mm(1, _C_L2, 29, 28, Bt[0:29, H + 0:H + 1], True, True)
            tanh_to(1, 28, H + 1)
            mm(2, _C_L3, 29, 12, Bt[0:29, H + 1:H + 2], True, True)
            tanh_to(2, 12, H + 2)
            mm(3, _C_L4, 13, 3, Bt[0:13, H + 2:H + 3], True, True)
            tanh_to(3, 3, H + 3)
            # --- L5: psi via replicated rows + correction col + h4 ---
            mm(4, _C_L5B, 5, 12, Bt[0:5, _C_EXT:_C_EXT + 1], True, False)
            mm(4, _C_L5C, 2, 12, Bt[0:2, _C_EXT2:_C_EXT2 + 1], False, False)
            mm(4, _C_L5A, 3, 12, Bt[0:3, H + 3:H + 4], False, True)
            tanh_to(4, 12, H + 4)
            # --- L6 ---
            mm(5, _C_L6B, 12, 12, Bt[0:12, H + 2:H + 3], True, False)
            mm(5, _C_L6A, 13, 12, Bt[0:13, H + 4:H + 5], False, True)
            tanh_to(5, 12, H + 5)
            # --- L7 ---
            mm(6, _C_L7B, 28, 28, Bt[0:28, H + 1:H + 2], True, False)
            mm(6, _C_L7A, 13, 28, Bt[0:13, H + 5:H + 6], False, True)
            tanh_to(6, 28, H + 6)
            # --- L8 transposed: result is a [1, 40] PSUM row. h7-part first
            # (it is the late dependency) so the PSUM group opens late. ---
            nc.tensor.matmul(ps8[:, :], Bt[0:28, H + 6:H + 7],
                             Bt[0:28, _C_L8A:_C_L8A + 40],
                             start=True, stop=False)
            nc.tensor.matmul(ps8[:, :], Bt[0:84, _C_V0:_C_V0 + 1],
                             Bt[0:84, _C_L8B:_C_L8B + 40],
                             start=False, stop=True)
            nc.scalar.activation(St[0:1, 8:48], ps8[0:1, 0:40],
                                 AF.Copy, bias=0.0, scale=1.0)
            # ACT issues the output DMA itself: no cross-engine hop, and the
            # single 160B descriptor replaces the 40-row scatter.
            nc.scalar.dma_start(out_dram[:, :], St[0:1, 8:48])

    nc.compile()
    _CACHE["nc"] = nc
    return nc


def _build_blk(inputs):
    blk = _build_w_block(inputs)
    _fill_x_block(blk, inputs["x"])
    return {"blk1": np.ascontiguousarray(blk[:, :_SPLIT]),
            "blk2": np.ascontiguousarray(blk[:, _SPLIT:])}


def kernel(**inputs) -> np.ndarray:
    nc = _build_bass()
    in_map = _build_blk(inputs)

    from concourse.bass_utils import run_bass_kernel_spmd

    res = run_bass_kernel_spmd(nc, [dict(in_map) for _ in range(8)],
                               core_ids=list(range(8)))
    out = np.asarray(res.results[0]["out"], F32).reshape(1, 40)
    return out
